# revision 2
# baseline (speedup 1.0000x reference)
"""AdaMoE layer on 8 Trainium2 NeuronCores — expert-parallel Bass/Tile kernel.

Strategy: each core k owns expert k and runs the dense FFN only for the
tokens its expert selects. Routing runs on the HOST in fp32: the host
gathers each expert's selected tokens into a 2432-token stream sorted by
descending routing weight (experts over capacity drop their
smallest-weight tokens), ships per-token routing weights, and each core
returns its weighted contribution in gathered order. The host
scatter-adds the 8 contributions (plus the closed-form sum_e w_e*b2_e
bias term) into the dense output. No device collectives, no device
gating.

Precision (validated by exact host-side emulation that reproduces the
measured HW rel err to 6 digits): subtiles 0-6 (896 top-weight tokens)
run fully in bf16; subtiles 7-18 run fully in fp8e4m3 DoubleRow (2x PE
throughput). Emulated rel err 0.018848 vs the 2e-2 gate. W1/W2 fp8
copies are pre-scaled by 32 (1/32 folded into the gelu input scale and
the shipped routing weights respectively). Contributions return in bf16.

Schedule (PE program order):
  FFN1 bf16 c0(256) c1(384) c2(256)   [W1 bf16 resident, scope 1]
  FFN2 bf16 j-outer phase, 7 subtiles [W2 streamed 8MB @~170GB/s,
                                       6 opsum banks + 1 hp-tag bank]
  FFN1/FFN2 fp8 c3 c4 c5 (512 each), software-pipelined
    [FFN1(c+1) emitted before FFN2(c) to hide the last-j activation
     latency at each chunk boundary]
W1-bf16's 64KB/partition SBUF is freed after c2 (scope close) and
reused for the fp8 W1/W2 copies, which stream in during the j-outer
phase. All device inputs are pre-transposed on host into SBUF tile
order so DMAs move contiguous partition lines.
"""

import numpy as np
import ml_dtypes

import concourse.bacc as bacc
import concourse.mybir as mybir
import concourse.tile as tile
from concourse.tile_rust import add_dep_helper
from concourse.bass_utils import run_bass_kernel_spmd

BF16 = ml_dtypes.bfloat16
F8E4 = ml_dtypes.float8_e4m3fn

B, S, D, FF, E = 2, 2048, 1024, 4096, 8
T = B * S
NCORES = 8
MAX_THRESHOLD = 0.125

P = 128            # SBUF partitions
SUB = 128          # tokens per PE output subtile
KD = D // P        # 8 contraction chunks over D
KF = FF // P       # 32 contraction chunks over FF
FP8SCALE = 32.0    # fp8 W1/W2 pre-scale (power of two)

# (width, ffn2_fp8, ffn1_fp8) per chunk, in descending routing-weight
# order. Stream = 2432 = 19*128; experts over capacity drop their
# smallest-weight tokens (fallback to CHUNKS_DENSE via DROP_FRAC guard).
CHUNKS = (
    (256, False, False), (384, False, False), (256, False, False),
    (512, True, True), (512, True, True), (512, True, True),
)
CHUNKS_DENSE = tuple(
    (c, False, False) for c in (256, 512, 512, 512, 512, 512, 512, 512, 256)
)
DROP_FRAC = 0.006                                # of summed routing weight

JB = 2             # streamed-W2 j-blocks per ring DMA (bf16-FFN2 phase)

dt = mybir.dt
Act = mybir.ActivationFunctionType
GELU_FUNC = Act.Gelu_apprx_tanh
DR = mybir.MatmulPerfMode.DoubleRow


def _build_gathered(n_cores=NCORES):
    """Specialized SPMD graph for the gathered CHUNKS config."""
    chunks = CHUNKS
    widths = [c for c, _, _ in chunks]
    tg = sum(widths)
    nsub_total = tg // SUB
    n16 = sum(w for w, f, _ in chunks if not f)     # 896 bf16 tokens
    nsub16 = n16 // SUB                              # 7
    g0s = [sum(widths[:c]) for c in range(len(chunks))]

    nc = bacc.Bacc(
        "TRN2",
        target_bir_lowering=False,
        debug=False,
        enable_asserts=True,
        num_devices=n_cores,
    )

    xT = nc.dram_tensor("xT", [P, KD * n16], dt.bfloat16, kind="ExternalInput")
    w1 = nc.dram_tensor("w1", [P, KF * KD * P], dt.bfloat16, kind="ExternalInput")
    w2 = nc.dram_tensor("w2", [P, KF * D], dt.bfloat16, kind="ExternalInput")
    xT8 = nc.dram_tensor(
        "xT8", [P, KD * (tg - n16)], dt.float8e4, kind="ExternalInput"
    )
    w1q = nc.dram_tensor("w1q", [P, KF * KD * P], dt.float8e4, kind="ExternalInput")
    w2q = nc.dram_tensor("w2q", [P, KF * D], dt.float8e4, kind="ExternalInput")
    b1t = nc.dram_tensor("b1t", [P, KF], dt.float32, kind="ExternalInput")
    wet = nc.dram_tensor("wet", [P, nsub_total], dt.float32, kind="ExternalInput")
    out_ext = nc.dram_tensor("out", [tg, D], dt.bfloat16, kind="ExternalOutput")

    w1_r = w1.ap().rearrange("p (j q) -> p j q", q=KD * P)     # [P, KF, KD*P]
    w2_r = w2.ap().rearrange("p (j d) -> p j d", d=D)          # [P, KF, D]
    w1q_r = w1q.ap().rearrange("p (a q) -> p a q", q=P)        # a = j*KD+kc
    w2q_r = w2q.ap().rearrange("p (j d) -> p j d", d=D)

    with tile.TileContext(nc) as tc:
        with (
            tc.tile_pool(name="const", bufs=1) as cpool,
            tc.tile_pool(name="x", bufs=2) as xpool,
            tc.tile_pool(name="h", bufs=1) as hpool,
            tc.tile_pool(name="h8", bufs=2) as h8pool,
            tc.tile_pool(name="w2s", bufs=6) as w2spool,
            tc.tile_pool(name="o", bufs=3) as opool,
            tc.tile_pool(name="hps", bufs=2, space="PSUM") as hpsum,
            tc.tile_pool(name="ops", bufs=6, space="PSUM") as opsum,
        ):
            b1_sb = cpool.tile([P, KF], dt.float32)
            nc.sync.dma_start(b1_sb[:], b1t.ap())
            we_sb = cpool.tile([P, nsub_total], dt.float32)
            nc.sync.dma_start(we_sb[:], wet.ap())

            # bf16 FFN1 output, all 7 bf16 subtiles (read by j-outer FFN2)
            ht_all = hpool.tile([P, KF, n16], dt.bfloat16, name="ht_all", tag="ht_all")

            def emit_out(ops_tile, idx, r0, dsl):
                # quarter-width staging keeps the osb pool at 1KB/partition
                for q in range(2):
                    osb = opool.tile([P, 256], dt.bfloat16, name="osb", tag="osb")
                    nc.vector.tensor_scalar_mul(
                        osb[:], ops_tile[:, q * 256 : (q + 1) * 256],
                        we_sb[:, idx : idx + 1],
                    )
                    qsl = slice(dsl.start + q * 256, dsl.start + (q + 1) * 256)
                    nc.sync.dma_start(out_ext.ap()[r0 : r0 + SUB, qsl], osb[:])

            # ---------- scope 1: bf16 FFN1 chunks, W1 resident ----------
            with tc.tile_pool(name="w1p", bufs=1) as w1pool:
                w1_sb = w1pool.tile([P, KF, KD * P], dt.bfloat16)
                w1_tail = []
                # j=0 head split by kc so the first matmul starts ASAP
                nc.sync.dma_start(w1_sb[:, 0, 0:P], w1_r[:, 0, 0:P])
                nc.sync.dma_start(w1_sb[:, 0, P : KD * P], w1_r[:, 0, P : KD * P])
                for j0, j1 in ((1, 2), (2, 4), (4, 8), (8, 12), (12, 16),
                               (16, 24), (24, 32)):
                    d = nc.sync.dma_start(w1_sb[:, j0:j1, :], w1_r[:, j0:j1, :])
                    if j1 - j0 >= 8:
                        w1_tail.append(d)

                # x chunk 0 in 4 kc-pair DMAs, chained (first lands fast)
                xt0 = xpool.tile([P, KD, widths[0]], dt.bfloat16, name="xt", tag="xt")
                prev = None
                for kq in range(0, KD, 2):
                    off = kq * widths[0]
                    d = nc.sync.dma_start(
                        xt0[:, kq : kq + 2, :],
                        xT.ap()[:, off : off + 2 * widths[0]].rearrange(
                            "p (k t) -> p k t", t=widths[0]
                        ),
                    )
                    if prev is not None:
                        add_dep_helper(d.ins, prev.ins, True, "xt0 chain")
                    prev = d

                # x chunks 1,2: behind the critical w1 tail, then chained
                xts = [xt0]
                x16_off = widths[0]
                for c in (1, 2):
                    cap = widths[c]
                    xt = xpool.tile([P, KD, cap], dt.bfloat16, name="xt", tag="xt")
                    d = nc.sync.dma_start(
                        xt[:],
                        xT.ap()[:, KD * x16_off : KD * (x16_off + cap)].rearrange(
                            "p (k t) -> p k t", t=cap
                        ),
                    )
                    for wd in w1_tail:
                        add_dep_helper(d.ins, wd.ins, True, "x after w1")
                    add_dep_helper(d.ins, prev.ins, True, "x chain")
                    prev = d
                    xts.append(xt)
                    x16_off += cap
                x_chain_tail = prev

                # FFN1: hT[f, t] = gelu(x @ W1 + b1).T  (j-inner, bf16)
                for c in range(3):
                    cap, g0 = widths[c], g0s[c]
                    for j in range(KF):
                        hp = hpsum.tile([P, cap], dt.float32, name="hp", tag="hp")
                        for kc in range(KD):
                            nc.tensor.matmul(
                                hp[:], w1_sb[:, j, kc * P : (kc + 1) * P],
                                xts[c][:, kc, :],
                                start=(kc == 0), stop=(kc == KD - 1),
                            )
                        nc.scalar.activation(
                            ht_all[:, j, g0 : g0 + cap], hp[:], GELU_FUNC,
                            bias=b1_sb[:, j : j + 1],
                        )

            # ---------- scope 2: fp8 weights reuse W1's SBUF ----------
            with tc.tile_pool(name="qp", bufs=1) as qpool:
                w1q_sb = qpool.tile([P, KF * KD, P], dt.float8e4)
                w2q_sb = qpool.tile([P, KF, D], dt.float8e4)
                # chained loads: w1q j-heads first (consumed j-progressively
                # by c3's FFN1), then w2q quarters. WAR on the freed w1pool
                # space delays the start until c2's last FFN1 matmul.
                prevq = None
                for j0, j1 in ((0, 2), (2, 4), (4, 8), (8, 16), (16, 32)):
                    d = nc.sync.dma_start(
                        w1q_sb[:, j0 * KD : j1 * KD, :], w1q_r[:, j0 * KD : j1 * KD, :]
                    )
                    if prevq is not None:
                        add_dep_helper(d.ins, prevq.ins, True, "w1q chain")
                    prevq = d
                JPW = KF // 4
                for i in range(4):
                    d = nc.sync.dma_start(
                        w2q_sb[:, i * JPW : (i + 1) * JPW, :],
                        w2q_r[:, i * JPW : (i + 1) * JPW, :],
                    )
                    add_dep_helper(d.ins, prevq.ins, True, "w2q chain")
                    prevq = d

                # xt8 prefetch chain (xpool bufs=2 paces slot reuse)
                xt8s = {}
                x8_off = 0
                prev_x8 = x_chain_tail
                def load_x8(c):
                    nonlocal x8_off, prev_x8
                    cap = widths[c]
                    xt = xpool.tile([P, KD, cap], dt.float8e4, name="xt", tag="xt")
                    d = nc.sync.dma_start(
                        xt[:],
                        xT8.ap()[:, KD * x8_off : KD * (x8_off + cap)].rearrange(
                            "p (k t) -> p k t", t=cap
                        ),
                    )
                    add_dep_helper(d.ins, prev_x8.ins, True, "x8 chain")
                    prev_x8 = d
                    x8_off += cap
                    xt8s[c] = xt

                # ---- FFN2-bf16 j-outer phase: 7 subtiles, streamed W2.
                # 6 opsum banks + 1 borrowed hp-tag bank; W2 ring chained
                # behind the w1 tail so prefetch fills during c1/c2.
                prev_w2s = None
                for half in range(2):
                    dsl = slice(half * 512, (half + 1) * 512)
                    opsA = []
                    for s in range(nsub16):
                        pool_, tg_ = (
                            (hpsum, "hp") if s == nsub16 - 1 else (opsum, "opsh")
                        )
                        opsA.append(
                            pool_.tile([P, 512], dt.float32, name="opsh", tag=tg_)
                        )
                    for g in range(0, KF, JB):
                        w2s = w2spool.tile(
                            [P, JB, 512], dt.bfloat16, name="w2s", tag="w2s"
                        )
                        dw = nc.sync.dma_start(w2s[:], w2_r[:, g : g + JB, dsl])
                        if prev_w2s is None:
                            for wd in w1_tail:
                                add_dep_helper(dw.ins, wd.ins, True, "w2s after w1")
                        else:
                            add_dep_helper(dw.ins, prev_w2s.ins, True, "w2s chain")
                        prev_w2s = dw
                        for jj in range(JB):
                            j = g + jj
                            for s in range(nsub16):
                                nc.tensor.matmul(
                                    opsA[s][:],
                                    ht_all[:, j, s * SUB : (s + 1) * SUB],
                                    w2s[:, jj, :],
                                    start=(j == 0), stop=(j == KF - 1),
                                    skip_group_check=True,
                                )
                    # hp-tag subtile first so c3's FFN1 hp slot frees early
                    for s in [nsub16 - 1] + list(range(nsub16 - 1)):
                        emit_out(opsA[s], s, s * SUB, dsl)

                # ---- fp8 chunks, software-pipelined ----
                def ffn1_fp8(c):
                    cap = widths[c]
                    ht = h8pool.tile(
                        [P, KF, cap], dt.float8e4, name="ht8", tag="ht8"
                    )
                    for j in range(KF):
                        hp = hpsum.tile([P, cap], dt.float32, name="hp", tag="hp")
                        for kc in range(0, KD, 2):
                            nc.tensor.matmul(
                                hp[:], w1q_sb[:, j * KD + kc : j * KD + kc + 2, :],
                                xt8s[c][:, kc : kc + 2, :],
                                start=(kc == 0), stop=(kc == KD - 2),
                                perf_mode=DR,
                            )
                        nc.scalar.activation(
                            ht[:, j, :], hp[:], GELU_FUNC,
                            bias=b1_sb[:, j : j + 1], scale=1.0 / FP8SCALE,
                        )
                    return ht

                def ffn2_fp8(c, ht):
                    cap, g0 = widths[c], g0s[c]
                    for s in range(cap // SUB):
                        tsl = slice(s * SUB, (s + 1) * SUB)
                        idx = g0 // SUB + s
                        r0 = g0 + s * SUB
                        for half in range(2):
                            dsl = slice(half * 512, (half + 1) * 512)
                            ops = opsum.tile(
                                [P, 512], dt.float32, name="opsh", tag="opsh"
                            )
                            for j in range(0, KF, 2):
                                nc.tensor.matmul(
                                    ops[:], ht[:, j : j + 2, tsl],
                                    w2q_sb[:, j : j + 2, dsl],
                                    start=(j == 0), stop=(j == KF - 2),
                                    perf_mode=DR,
                                )
                            emit_out(ops, idx, r0, dsl)

                load_x8(3)
                load_x8(4)
                ht3 = ffn1_fp8(3)
                load_x8(5)
                ht4 = ffn1_fp8(4)
                ffn2_fp8(3, ht3)
                ht5 = ffn1_fp8(5)
                ffn2_fp8(4, ht4)
                ffn2_fp8(5, ht5)

    nc.compile()
    return nc


def _build_dense(chunks=CHUNKS_DENSE, n_cores=NCORES):
    """Generic fallback graph (dense, all-bf16, W2 resident)."""
    widths = [c for c, _, _ in chunks]
    tg = sum(widths)
    nsub_total = tg // SUB
    any_f2 = any(f for _, f, _ in chunks)
    any_f1 = any(f for _, _, f in chunks)
    nx16 = sum(w for w, _, f in chunks if not f)
    g0s = [sum(widths[:c]) for c in range(len(chunks))]
    assert not any_f1 and not any_f2

    nc = bacc.Bacc(
        "TRN2",
        target_bir_lowering=False,
        debug=False,
        enable_asserts=True,
        num_devices=n_cores,
    )

    xT = nc.dram_tensor("xT", [P, KD * nx16], dt.bfloat16, kind="ExternalInput")
    w1 = nc.dram_tensor("w1", [P, KF * KD * P], dt.bfloat16, kind="ExternalInput")
    w2 = nc.dram_tensor("w2", [P, KF * D], dt.bfloat16, kind="ExternalInput")
    b1t = nc.dram_tensor("b1t", [P, KF], dt.float32, kind="ExternalInput")
    wet = nc.dram_tensor("wet", [P, nsub_total], dt.float32, kind="ExternalInput")
    out_ext = nc.dram_tensor("out", [tg, D], dt.bfloat16, kind="ExternalOutput")

    w1_r = w1.ap().rearrange("p (j q) -> p j q", q=KD * P)
    w2_r = w2.ap().rearrange("p (j d) -> p j d", d=D)

    with tile.TileContext(nc) as tc:
        with (
            tc.tile_pool(name="const", bufs=1) as cpool,
            tc.tile_pool(name="x", bufs=2) as xpool,
            tc.tile_pool(name="h", bufs=1) as hpool,
            tc.tile_pool(name="o", bufs=3) as opool,
            tc.tile_pool(name="hps", bufs=2, space="PSUM") as hpsum,
            tc.tile_pool(name="ops", bufs=6, space="PSUM") as opsum,
        ):
            b1_sb = cpool.tile([P, KF], dt.float32)
            nc.sync.dma_start(b1_sb[:], b1t.ap())
            we_sb = cpool.tile([P, nsub_total], dt.float32)
            nc.sync.dma_start(we_sb[:], wet.ap())

            xt0 = xpool.tile([P, KD, widths[0]], dt.bfloat16, tag="xt")
            for kq in (0, KD // 2):
                off = kq * widths[0]
                nc.sync.dma_start(
                    xt0[:, kq : kq + KD // 2, :],
                    xT.ap()[:, off : off + (KD // 2) * widths[0]].rearrange(
                        "p (k t) -> p k t", t=widths[0]
                    ),
                )

            w1_sb = cpool.tile([P, KF, KD * P], dt.bfloat16)
            w1_tail = []
            for j0, j1 in ((0, 1), (1, 2), (2, 4), (4, 8), (8, 12), (12, 16),
                           (16, 24), (24, 32)):
                d = nc.sync.dma_start(w1_sb[:, j0:j1, :], w1_r[:, j0:j1, :])
                if j1 - j0 >= 8:
                    w1_tail.append(d)

            JPW = KF // 4
            w2_sb = cpool.tile([P, KF, D], dt.bfloat16)
            w2_dmas = []
            for i in range(4):
                d = nc.sync.dma_start(
                    w2_sb[:, i * JPW : (i + 1) * JPW, :],
                    w2_r[:, i * JPW : (i + 1) * JPW, :],
                )
                for pd in w1_tail:
                    add_dep_helper(d.ins, pd.ins, True, "w2 after w1 tails")
                w2_dmas.append(d)

            def w1_ap(kc, j):
                return w1_sb[:, j, kc * P : (kc + 1) * P]

            def emit_out(ops_tile, idx, r0, dsl):
                for q in range(2):
                    osb = opool.tile([P, 256], dt.bfloat16, name="osb", tag="osb")
                    nc.vector.tensor_scalar_mul(
                        osb[:], ops_tile[:, q * 256 : (q + 1) * 256],
                        we_sb[:, idx : idx + 1],
                    )
                    qsl = slice(dsl.start + q * 256, dsl.start + (q + 1) * 256)
                    nc.sync.dma_start(out_ext.ap()[r0 : r0 + SUB, qsl], osb[:])

            prev_xt_dma = None
            x16_off = 0
            for c, (cap, f2, f1) in enumerate(chunks):
                g0 = g0s[c]
                if c == 0:
                    xt = xt0
                else:
                    xt = xpool.tile([P, KD, cap], dt.bfloat16, name="xt", tag="xt")
                    d = nc.sync.dma_start(
                        xt[:],
                        xT.ap()[:, KD * x16_off : KD * (x16_off + cap)].rearrange(
                            "p (k t) -> p k t", t=cap
                        ),
                    )
                    for wd in w2_dmas:
                        add_dep_helper(d.ins, wd.ins, True, "x after weights")
                    if prev_xt_dma is not None:
                        add_dep_helper(d.ins, prev_xt_dma.ins, True, "x chain")
                    prev_xt_dma = d
                x16_off += cap

                ht = hpool.tile([P, KF, cap], dt.bfloat16, name="ht", tag="ht8")
                for j in range(KF):
                    hp = hpsum.tile([P, cap], dt.float32, name="hp", tag="hp")
                    for kc in range(KD):
                        nc.tensor.matmul(
                            hp[:], w1_ap(kc, j), xt[:, kc, :],
                            start=(kc == 0), stop=(kc == KD - 1),
                        )
                    nc.scalar.activation(
                        ht[:, j, :], hp[:], GELU_FUNC, bias=b1_sb[:, j : j + 1],
                    )

                for s in range(cap // SUB):
                    tsl = slice(s * SUB, (s + 1) * SUB)
                    idx = g0 // SUB + s
                    r0 = g0 + s * SUB
                    for half in range(2):
                        dsl = slice(half * 512, (half + 1) * 512)
                        ops = opsum.tile(
                            [P, 512], dt.float32, name="opsh", tag="opsh"
                        )
                        for j in range(KF):
                            nc.tensor.matmul(
                                ops[:], ht[:, j, tsl], w2_sb[:, j, dsl],
                                start=(j == 0), stop=(j == KF - 1),
                            )
                        emit_out(ops, idx, r0, dsl)

    nc.compile()
    return nc


_NC_CACHE = {}


def _get_nc(chunks=CHUNKS, n_cores=NCORES):
    key = (tuple(chunks), n_cores)
    if key not in _NC_CACHE:
        if tuple(chunks) == CHUNKS:
            _NC_CACHE[key] = _build_gathered(n_cores)
        else:
            _NC_CACHE[key] = _build_dense(chunks, n_cores)
    return _NC_CACHE[key]


def _gating(x, wg, bg, wt, bt):
    """fp32 routing: selection mask and normalized per-token weights."""
    logits = x @ np.concatenate([wg, wt], axis=1) + np.concatenate(
        [bg, bt]
    ).astype(np.float32)
    lg = logits[:, :E]
    lg = lg - lg.max(-1, keepdims=True)
    ex = np.exp(lg)
    gate = ex / ex.sum(-1, keepdims=True)
    thr = (1.0 / (1.0 + np.exp(-logits[:, E : E + 1]))) * MAX_THRESHOLD
    adapted = gate - thr
    sel = adapted >= 0
    w = np.where(sel, adapted, 0.0)
    s = w.sum(-1, keepdims=True)
    s[s == 0] = 1.0
    w = (w / s).astype(np.float32)
    return sel, w


def _x_blocks(xg, widths, dtype):
    """[n, D] f32 -> [P, KD*n] in per-chunk [kc, t] block order."""
    n = sum(widths)
    outb = np.empty((P, KD * n), dtype=dtype)
    g0 = 0
    for cap in widths:
        blk = xg[g0 : g0 + cap].T.reshape(KD, P, cap).transpose(1, 0, 2)
        outb[:, KD * g0 : KD * (g0 + cap)] = blk.reshape(P, KD * cap)
        g0 += cap
    return outb


def kernel(inputs, Wg, bg, Wt, bt, W1, b1, W2, b2, _trace=False):
    x = np.ascontiguousarray(np.asarray(inputs, dtype=np.float32).reshape(-1, D))
    sel, w = _gating(
        x,
        np.asarray(Wg, dtype=np.float32), np.asarray(bg, dtype=np.float32),
        np.asarray(Wt, dtype=np.float32), np.asarray(bt, dtype=np.float32),
    )
    W1 = np.asarray(W1)
    W2 = np.asarray(W2)
    b1 = np.asarray(b1)

    # Experts over capacity drop their smallest-weight tokens; if that
    # would discard a non-trivial share of routed weight, process densely.
    cap = sum(c for c, _, _ in CHUNKS)
    rows_try, dropped_w = [], 0.0
    for k in range(NCORES):
        rows = np.flatnonzero(sel[:, k])
        if len(rows) > cap:
            order = np.argsort(w[rows, k])
            dropped_w += float(w[rows, k][order[: len(rows) - cap]].sum())
            rows = rows[order[len(rows) - cap :]]
        rows_try.append(rows[np.argsort(w[rows, k])[::-1]])  # descending w
    gathered = dropped_w <= DROP_FRAC * max(float(w.sum()), 1.0)
    chunks = CHUNKS if gathered else CHUNKS_DENSE
    widths = [c for c, _, _ in chunks]
    tg = sum(widths)
    nsub = tg // SUB
    subf8 = []
    x16w, x8w = [], []
    for capc, f2, f1 in chunks:
        subf8 += [f2] * (capc // SUB)
        (x8w if f1 else x16w).append(capc)
    any_f1 = len(x8w) > 0
    any_f2 = any(f for _, f, _ in chunks)

    in_maps = []
    rows_all = []
    for k in range(NCORES):
        rows = rows_try[k] if gathered else np.arange(T)
        rows_all.append(rows)
        xg = np.zeros((tg, D), dtype=np.float32)
        xg[: len(rows)] = x[rows]
        wek = np.zeros((tg,), dtype=np.float32)
        wek[: len(rows)] = w[rows, k]
        for si in range(nsub):
            if subf8[si]:
                wek[si * SUB : (si + 1) * SUB] /= FP8SCALE
        n16 = sum(x16w)
        w1d = (
            W1[k].astype(BF16).reshape(KD, P, KF, P)
            .transpose(1, 2, 0, 3).reshape(P, KF * KD * P)
        )
        m = {
            "xT": _x_blocks(xg[:n16], x16w, BF16),
            "w1": np.ascontiguousarray(w1d),
            "w2": np.ascontiguousarray(
                W2[k].astype(BF16).reshape(KF, P, D)
                .transpose(1, 0, 2).reshape(P, KF * D)
            ),
            "b1t": np.ascontiguousarray(
                b1[k].astype(np.float32).reshape(KF, P).T
            ),
            "wet": np.ascontiguousarray(wek.reshape(nsub, SUB).T),
        }
        if any_f1:
            m["xT8"] = _x_blocks(xg[n16:], x8w, F8E4)
            m["w1q"] = np.ascontiguousarray(
                (FP8SCALE * W1[k]).astype(F8E4).reshape(KD, P, KF, P)
                .transpose(1, 2, 0, 3).reshape(P, KF * KD * P)
            )
        if any_f2:
            m["w2q"] = np.ascontiguousarray(
                (FP8SCALE * W2[k]).astype(F8E4).reshape(KF, P, D)
                .transpose(1, 0, 2).reshape(P, KF * D)
            )
        in_maps.append(m)

    nc = _get_nc(chunks)
    res = run_bass_kernel_spmd(
        nc, in_maps, core_ids=list(range(NCORES)), trace=_trace,
    )
    kernel._last_results = res

    # combine: closed-form bias term + scatter-add of core contributions
    out = w @ np.asarray(b2, dtype=np.float32)          # [T, D]
    for k in range(NCORES):
        r = np.asarray(res.results[k]["out"]).reshape(tg, D).astype(np.float32)
        rows = rows_all[k]
        out[rows] += r[: len(rows)]
    return out.reshape(B, S, D).astype(np.float32)
